# revision 15
# baseline (speedup 1.0000x reference)
"""Trainium2 Bass kernel for causal self-attention (B=2, S=2048, D=1024, H=16).

Sharding: 8 cores = 2 (batch) x 4 (head groups of 4 heads) — data parallel on
batch, tensor parallel on heads. Each core computes, for its batch b and its
4 heads (256 of the 1024 model dims):

  qT/kT = Wq_slice^T x^T            transposed layouts [head_dim, seq], fp16
  v     = x Wv_slice                natural layout [seq, head_dim], fp16
  per head pair (2 heads share the 128 partitions):
    scoresT[kv, q] blocks on PE (two row-packed K=64 matmuls),
    exp on ACT (psum -> fp16 sbuf), causal mask multiply on DVE (fp16 2x),
    P^T V + replicated ones-row denominators on PE (col-packed M=64),
    normalize: reciprocal_approx_fast + one tensor_mul.
  o_partial = Wo_slice^T attnT      [1024, seq] fp16 partials

Host: feeds x^T (nt-major contiguous) and fp16 weight slices, sums the
fp16 partials per batch, transposes, adds bo.

All matmuls run in fp16 (1 cyc/row on PE) with fp32 PSUM accumulation;
softmax scale 1/sqrt(64) is folded into Wq on the host. Input DMAs are
consolidated (one per weight, one per 512-seq column block of x^T) and
priority-ordered so the first projections start ~6us in; the causal mask is
sliced out of a single [128, 896] staircase master. O-projection for the
last 512 q columns is split per head-group so one group's 8 matmuls ride as
fillers inside the final attention call; a few dummy matmuls at the top
warm the PE clock (DVFS p-state) while the first DMAs land.
"""

import numpy as np

import concourse.bacc as bacc
import concourse.tile as tile
from concourse import mybir
from concourse.bass_utils import run_bass_kernel_spmd

B, S, D, H = 2, 2048, 1024, 16
HD = D // H          # 64
P = 128
NCORES = 8
GROUPS = 4           # head groups (tensor parallel)
HPG = H // GROUPS    # 4 heads per group
CD = HPG * HD        # 256 local head dims per core
QT = 512             # q tile (matmul free dim)
KT = 128             # kv tile (psum partition dim)
NQT = S // QT        # 4
NKT = S // KT        # 16
KD = D // P          # 8 contraction tiles over the model dim
MW = 3 * KT + QT     # 896: master mask width

F32 = mybir.dt.float32
F16 = mybir.dt.float16
EXP = mybir.ActivationFunctionType.Exp

_NC_CACHE = {}


def _build_nc():
    if "nc" in _NC_CACHE:
        return _NC_CACHE["nc"]
    nc = bacc.Bacc()
    xt = nc.declare_dram_parameter("xt", [NQT, P, KD, QT], F16, isOutput=False)
    wq = nc.declare_dram_parameter("wq", [P, KD, CD], F16, isOutput=False)
    wk = nc.declare_dram_parameter("wk", [P, KD, CD], F16, isOutput=False)
    wv = nc.declare_dram_parameter("wv", [P, KD, CD], F16, isOutput=False)
    wo = nc.declare_dram_parameter("wo", [P, 2, D], F16, isOutput=False)
    bq = nc.declare_dram_parameter("bq", [CD], F32, isOutput=False)
    bk = nc.declare_dram_parameter("bk", [CD], F32, isOutput=False)
    bv = nc.declare_dram_parameter("bv", [HPG, HD], F32, isOutput=False)
    msk = nc.declare_dram_parameter("msk", [P, MW], F16, isOutput=False)
    # o partials: t = 0..2 with both groups summed on-chip; t = 3 kept
    # per-group so the o-projection can overlap the last attention call
    otc = nc.declare_dram_parameter("otc", [NQT - 1, KD, P, QT], F16,
                                    isOutput=True)
    otg = nc.declare_dram_parameter("otg", [2, KD, P, QT], F16, isOutput=True)

    import concourse.bass as bass

    with tile.TileContext(nc) as tc:
        with tc.tile_pool(name="consts", bufs=1) as consts, \
             tc.tile_pool(name="work", bufs=3) as work, \
             tc.tile_pool(name="ps_s", bufs=2, space="PSUM") as ps_s, \
             tc.tile_pool(name="ps_av", bufs=2, space="PSUM") as ps_av, \
             tc.tile_pool(name="ps_po", bufs=2, space="PSUM") as ps_po:

            # ---- constant / persistent SBUF tensors ----
            xt_sb = consts.tile([P, KD, S], F16)
            wq_sb = consts.tile([P, KD, CD], F16)
            wk_sb = consts.tile([P, KD, CD], F16)
            wv_sb = consts.tile([P, KD, CD], F16)
            wo_sb = consts.tile([P, 2, D], F16)
            bq_sb = consts.tile([P, 2], F32)
            bk_sb = consts.tile([P, 2], F32)
            bv_sb = consts.tile([P, HPG, HD], F32)
            msk_sb = consts.tile([P, MW], F16)
            qT_sb = consts.tile([P, 2, S], F16)
            kT_sb = consts.tile([P, 2, S], F16)
            v2_sb = consts.tile([P, NKT, 2, 3 * HD], F16)
            aT_sb = consts.tile([P, 2, NQT, QT], F16)
            dumm_sb = consts.tile([P, QT], F16)

            # ---- PE clock pre-warm: the PE DVFS p-state needs ~3us of
            # continuous execution to reach full clock; run throwaway
            # matmuls on an un-DMA'd tile while the input DMAs stream ----
            nc.vector.memset(dumm_sb, 0.0)
            ps_warm = ps_po.tile([P, QT], F32, tag="po", name="ps_warm")
            for w in range(6):
                nc.tensor.matmul(ps_warm, dumm_sb[:, 0:P], dumm_sb,
                                 start=(w == 0), stop=(w == 5))

            # ---- input DMAs: consolidated, priority-ordered so the first
            # projections (wq + x columns 0:512) can start ~6us in.  Small
            # tensors ride the gpsimd queue in parallel with the sync queue.
            nc.gpsimd.dma_start(out=bq_sb, in_=bq[:].rearrange("(m p) -> p m", p=P))
            nc.gpsimd.dma_start(out=bk_sb, in_=bk[:].rearrange("(m p) -> p m", p=P))
            bv_ap = bv[:, :]
            bv_bc = bass.AP(tensor=bv_ap.tensor, offset=bv_ap.offset,
                            ap=[[0, P]] + list(bv_ap.ap))
            nc.gpsimd.dma_start(out=bv_sb, in_=bv_bc)
            nc.gpsimd.dma_start(out=msk_sb, in_=msk[:, :])
            nc.vector.memset(v2_sb[:, :, :, HD:2 * HD], 1.0)
            # first-needed tensors split in halves and spread across four
            # issue queues so they transfer concurrently
            h = KD // 2
            nc.sync.dma_start(out=wq_sb[:, 0:h, :], in_=wq[:, 0:h, :])
            nc.scalar.dma_start(out=xt_sb[:, 0:h, 0:QT], in_=xt[0][:, 0:h, :])
            nc.sync.dma_start(out=wq_sb[:, h:KD, :], in_=wq[:, h:KD, :])
            nc.scalar.dma_start(out=xt_sb[:, h:KD, 0:QT], in_=xt[0][:, h:KD, :])
            nc.sync.dma_start(out=wk_sb, in_=wk[:, :, :])
            nc.scalar.dma_start(out=wv_sb, in_=wv[:, :, :])
            nc.scalar.dma_start(out=xt_sb[:, :, QT:2 * QT], in_=xt[1])
            nc.sync.dma_start(out=wo_sb, in_=wo[:, :, :])
            nc.gpsimd.dma_start(out=xt_sb[:, :, 2 * QT:3 * QT], in_=xt[2])
            nc.gpsimd.dma_start(out=xt_sb[:, :, 3 * QT:4 * QT], in_=xt[3])

            # ---- helpers ----
            def proj_qk(w_sb, b_sb, dst, mt, nts):
                pss = [ps_po.tile([P, QT], F32, tag="po", name=f"ps_qk{j}")
                       for j in range(len(nts))]
                for kt in range(KD):
                    lhs = w_sb[:, kt, mt * P:(mt + 1) * P]
                    for j, nt in enumerate(nts):
                        nc.tensor.matmul(
                            pss[j], lhs,
                            xt_sb[:, kt, nt * QT:(nt + 1) * QT],
                            start=(kt == 0), stop=(kt == KD - 1))
                for j, nt in enumerate(nts):
                    nc.vector.tensor_scalar_add(
                        dst[:, mt, nt * QT:(nt + 1) * QT], pss[j],
                        b_sb[:, mt:mt + 1])

            def proj_v(jt0, jt1):
                for jt in range(jt0, jt1):
                    ps = ps_po.tile([P, QT], F32, tag="po", name="ps_v")
                    for kt in range(KD):
                        nc.tensor.matmul(
                            ps[:, :CD], xt_sb[:, kt, jt * P:(jt + 1) * P],
                            wv_sb[:, kt, :],
                            start=(kt == 0), stop=(kt == KD - 1))
                    psh = ps[:, :CD].rearrange("p (h d) -> p h d", h=HPG)
                    # even heads -> cols 0:64, odd heads -> cols 128:192
                    nc.vector.tensor_add(
                        v2_sb[:, jt, :, 0:HD], psh[:, 0::2, :], bv_sb[:, 0::2, :])
                    nc.vector.tensor_add(
                        v2_sb[:, jt, :, 2 * HD:3 * HD], psh[:, 1::2, :],
                        bv_sb[:, 1::2, :])

            def attention(t, g, fillers=()):
                n_kv = 4 * (t + 1)
                # bank A: rows 0:64 = attn h(2g), rows 64:128 = denom h(2g)
                # bank B: rows 0:64 = denom h(2g+1), rows 64:128 = attn h(2g+1)
                av_a = ps_av.tile([P, QT], F32, tag="avden", name="av_a")
                av_b = ps_av.tile([P, QT], F32, tag="avden", name="av_b")

                def vr(kv):
                    r = kv - 4 * t
                    return r, (KT * r if r >= 1 else 0)   # first valid q col

                def scores(kv):
                    r, v0 = vr(kv)
                    s = ps_s.tile([P, 2 * QT], F32, tag="s", name="s")
                    for idx in range(2):
                        p0 = 64 * idx
                        nc.tensor.matmul(
                            s[:, idx * QT + v0:(idx + 1) * QT],
                            kT_sb[p0:p0 + 64, g, kv * KT:(kv + 1) * KT],
                            qT_sb[p0:p0 + 64, g, t * QT + v0:(t + 1) * QT],
                            start=True, stop=True)
                    return s

                # software-pipelined: the PE issues scores(kv+1) BEFORE the
                # exp/mask-dependent AV(kv) so it never idles waiting on the
                # ACT->DVE round trip.  The slot-0 filler may feed scores(0)
                # (k-projection just in time), so it runs before the prologue.
                if fillers and fillers[0] is not None:
                    fillers[0]()
                s = scores(0)
                for kv in range(n_kv):
                    r, v0 = vr(kv)
                    p_t = work.tile([P, 2 * QT], F16, tag="pt", name="p_t")
                    if r < 1:
                        nc.scalar.activation(p_t, s, EXP)
                    else:
                        for idx in range(2):
                            sl = slice(idx * QT + v0, (idx + 1) * QT)
                            nc.scalar.activation(p_t[:, sl], s[:, sl], EXP)
                    if r >= 0:
                        # causal mask for diagonal offset r: slice the
                        # [128, 896] staircase master at column 128*(3-r)
                        mb = KT * (3 - r)
                        for idx in range(2):
                            sl = slice(idx * QT + v0, (idx + 1) * QT)
                            nc.vector.tensor_mul(
                                p_t[:, sl], p_t[:, sl],
                                msk_sb[:, mb + v0:mb + QT])
                    if kv + 1 < n_kv:
                        s = scores(kv + 1)
                    if 0 < kv < len(fillers) and fillers[kv] is not None:
                        fillers[kv]()
                    for idx, bank in ((0, av_a), (1, av_b)):
                        rhs = p_t[:, idx * QT + v0:(idx + 1) * QT]
                        lhsT = v2_sb[:, kv, g, HD * idx:HD * idx + 2 * HD]
                        nc.tensor.matmul(
                            bank[:, v0:], lhsT, rhs,
                            start=(kv == 0), stop=(kv == n_kv - 1))
                # normalize: aT = av / den, denominators shifted across
                # partition halves via a small SBUF->SBUF DMA
                rca = work.tile([P, QT], F32, tag="rca", name="rca")
                rcb = work.tile([P, QT], F32, tag="rcb", name="rcb")
                rc2 = work.tile([P, QT], F32, tag="rc2", name="rc2")
                nc.vector.reciprocal_approx_fast(rca, av_a)
                nc.vector.reciprocal_approx_fast(rcb, av_b)
                nc.sync.dma_start(out=rc2[0:64, :], in_=rca[64:128, :])
                nc.sync.dma_start(out=rc2[64:128, :], in_=rcb[0:64, :])
                nc.vector.tensor_mul(aT_sb[0:64, g, t, :], av_a[0:64, :],
                                     rc2[0:64, :])
                nc.vector.tensor_mul(aT_sb[64:128, g, t, :], av_b[64:128, :],
                                     rc2[64:128, :])

            # o-projection into fp16 staging, one DMA per staged tile.
            # t = 0..2: both groups accumulated; t = 3: per-group partials.
            og_st = {}

            def og_unit(t, m0, n_mt=2, g=None, dma=False, copy_alt=False):
                key = (t, g)
                if key not in og_st:
                    og_st[key] = work.tile([P, KD, QT], F16, tag="ogst",
                                           name=f"og{t}{g}")
                st = og_st[key]
                for mt_e in range(m0, m0 + n_mt):
                    ps = ps_po.tile([P, QT], F32, tag="po", name="ps_o")
                    if g is None:
                        for gg in range(2):
                            nc.tensor.matmul(
                                ps, wo_sb[:, gg, mt_e * P:(mt_e + 1) * P],
                                aT_sb[:, gg, t, :],
                                start=(gg == 0), stop=(gg == 1))
                    else:
                        nc.tensor.matmul(
                            ps, wo_sb[:, g, mt_e * P:(mt_e + 1) * P],
                            aT_sb[:, g, t, :], start=True, stop=True)
                    # copy_alt alternates psum->sbuf copies between the
                    # scalar and vector engines (tail: both near-idle)
                    if copy_alt and mt_e % 2 == 0:
                        nc.scalar.copy(st[:, mt_e, :], ps)
                    else:
                        nc.vector.tensor_copy(st[:, mt_e, :], ps)
                if dma:
                    dst = otc[t] if g is None else otg[g]
                    nc.sync.dma_start(
                        out=dst.rearrange("m p c -> p m c"), in_=st)

            # ---- filler-interleaved schedule: the PE stream alternates
            # one projection/o-proj chunk per attention kv-iteration so the
            # in-order PE queue never sits on a block of non-attention work
            # while ACT starves ----
            def fq(mt, nt):
                return lambda: proj_qk(wq_sb, bq_sb, qT_sb, mt, [nt])

            def fk(mt, nt):
                return lambda: proj_qk(wk_sb, bk_sb, kT_sb, mt, [nt])

            def fv(jt):
                return lambda: proj_v(jt, jt + 1)

            def fo(t, m0, g=None, dma=False):
                return lambda: og_unit(t, m0, g=g, dma=dma)

            # prefix: just enough for att(0,0)
            proj_qk(wq_sb, bq_sb, qT_sb, 0, [0])
            proj_qk(wk_sb, bk_sb, kT_sb, 0, [0])
            proj_v(0, 1)
            attention(0, 0, [fv(1), fv(2), fv(3), fq(1, 0)])
            attention(0, 1, [fk(1, 0), fq(0, 1), fk(0, 1), fq(1, 1)])
            attention(1, 0, [fv(4), fo(0, 0), fo(0, 2), fo(0, 4),
                             fo(0, 6, dma=True), fv(5), fv(6), fv(7)])
            attention(1, 1, [fk(1, 1), fq(0, 2), fk(0, 2), fq(1, 2),
                             fk(1, 2)])
            attention(2, 0, [fv(8), fo(1, 0), fo(1, 2), fo(1, 4),
                             fo(1, 6, dma=True), fv(9), fv(10), fv(11)])
            attention(2, 1, [fq(0, 3), fk(0, 3), fv(12), fv(13), fv(14),
                             fv(15), fq(1, 3), fk(1, 3)])
            attention(3, 0, [None, None, fo(2, 0), None, None, fo(2, 2),
                             None, None, fo(2, 4), None, None,
                             fo(2, 6, dma=True)])
            attention(3, 1, [None, None, None, None, None, None,
                             fo(3, 0, g=0)])
            # tail: og(3,0)'s remaining units run right after the last AV so
            # the PE stays busy (and at full clock) while att(3,1)'s
            # normalize chain (recip -> cross-partition DMA -> mul) drains;
            # og(3,1) then alternates its psum->sbuf copies between the
            # scalar and vector engines with a finer-grained output DMA
            og_unit(3, 2, n_mt=6, g=0, dma=True, copy_alt=True)
            og_unit(3, 0, n_mt=4, g=1, copy_alt=True)
            nc.sync.dma_start(out=otg[1, 0:4].rearrange("m p c -> p m c"),
                              in_=og_st[(3, 1)][:, 0:4, :])
            og_unit(3, 4, n_mt=2, g=1, copy_alt=True)
            nc.sync.dma_start(out=otg[1, 4:6].rearrange("m p c -> p m c"),
                              in_=og_st[(3, 1)][:, 4:6, :])
            og_unit(3, 6, n_mt=2, g=1, copy_alt=True)
            nc.sync.dma_start(out=otg[1, 6:8].rearrange("m p c -> p m c"),
                              in_=og_st[(3, 1)][:, 6:8, :])

    nc.compile()
    _NC_CACHE["nc"] = nc
    return nc


def _make_mask():
    # staircase master: M[p, u] = 1 iff p <= u - 384; mask for diagonal
    # offset r (valid iff p <= c - 128 r, c in [0,512)) = M[:, 128(3-r)+c]
    pp = np.arange(P)[:, None]
    uu = np.arange(MW)[None, :]
    return (pp <= uu - 3 * KT).astype(np.float16)


def _in_maps(x, Wq, bq, Wk, bk, Wv, bv, Wo):
    scale = np.float32(1.0 / np.sqrt(HD))
    mask = _make_mask()
    maps = []
    for core in range(NCORES):
        b, g = divmod(core, GROUPS)
        csl = slice(g * CD, (g + 1) * CD)
        xT = np.ascontiguousarray(x[b].T).astype(np.float16)        # [D, S]
        xt4 = np.ascontiguousarray(
            xT.reshape(KD, P, NQT, QT).transpose(2, 1, 0, 3))
        wq_h = np.ascontiguousarray(
            (np.asarray(Wq[:, csl]) * scale).astype(np.float16)
            .reshape(KD, P, CD).transpose(1, 0, 2))
        wk_h = np.ascontiguousarray(
            np.asarray(Wk[:, csl]).astype(np.float16)
            .reshape(KD, P, CD).transpose(1, 0, 2))
        wv_h = np.ascontiguousarray(
            np.asarray(Wv[:, csl]).astype(np.float16)
            .reshape(KD, P, CD).transpose(1, 0, 2))
        wo_h = np.ascontiguousarray(
            np.asarray(Wo[csl, :]).astype(np.float16)
            .reshape(2, P, D).transpose(1, 0, 2))
        maps.append({
            "xt": xt4,
            "wq": wq_h,
            "wk": wk_h,
            "wv": wv_h,
            "wo": wo_h,
            "bq": np.ascontiguousarray(bq[csl] * scale).astype(np.float32),
            "bk": np.ascontiguousarray(bk[csl]).astype(np.float32),
            "bv": np.ascontiguousarray(bv[csl]).reshape(HPG, HD).astype(np.float32),
            "msk": mask,
        })
    return maps


def _ot_to_oT(otc, otg):
    """otc [3, KD, P, QT] + otg [2, KD, P, QT] fp16 -> [D, S] fp32 partial."""
    a = np.asarray(otc).reshape(NQT - 1, KD, P, QT).astype(np.float32)
    g = np.asarray(otg).reshape(2, KD, P, QT).astype(np.float32).sum(axis=0)
    full = np.concatenate([a, g[None]], axis=0)     # [NQT, KD, P, QT]
    return full.transpose(1, 2, 0, 3).reshape(D, S)


def kernel_with_results(x, Wq, bq, Wk, bk, Wv, bv, Wo, bo, trace=False):
    nc = _build_nc()
    maps = _in_maps(x, Wq, bq, Wk, bk, Wv, bv, Wo)
    kwargs = {}
    if trace:
        kwargs = dict(trace=True, trace_cores=[0])
    res = run_bass_kernel_spmd(nc, maps, core_ids=list(range(NCORES)), **kwargs)
    out = np.zeros((B, S, D), dtype=np.float32)
    for b in range(B):
        acc = np.zeros((D, S), dtype=np.float32)
        for g in range(GROUPS):
            r = res.results[b * GROUPS + g]
            acc += _ot_to_oT(r["otc"], r["otg"])
        out[b] = acc.T + np.asarray(bo, dtype=np.float32)[None, :]
    return out, res


def kernel(x, Wq, bq, Wk, bk, Wv, bv, Wo, bo):
    out, _ = kernel_with_results(x, Wq, bq, Wk, bk, Wv, bv, Wo, bo, trace=False)
    return out


# revision 20
# speedup vs baseline: 1.2031x; 1.2031x over previous
"""Trainium2 Bass kernel for causal self-attention (B=2, S=2048, D=1024, H=16).

Sharding: 8 cores = 2 (batch) x 4 (head groups of 4 heads) — data parallel on
batch, tensor parallel on heads. Each core computes, for its batch b and its
4 heads (256 of the 1024 model dims):

  qT/kT = Wq_slice^T x^T            transposed layouts [head_dim, seq], fp16
  v     = x Wv_slice                natural layout [seq, head_dim], fp16
  per head pair (2 heads share the 128 partitions):
    scoresT[kv, q] blocks on PE (two row-packed K=64 matmuls),
    exp on ACT (psum -> fp16 sbuf), causal mask multiply on DVE (fp16 2x),
    P^T V + replicated ones-row denominators on PE (col-packed M=64),
    normalize: reciprocal_approx_fast + one tensor_mul.
  o_partial = Wo_slice^T attnT      [1024, seq] fp16 partials

Host: feeds x^T (nt-major contiguous) and fp16 weight slices, sums the
fp16 partials per batch, transposes, adds bo.

All matmuls run in fp16 (1 cyc/row on PE) with fp32 PSUM accumulation;
softmax scale 1/sqrt(64) is folded into Wq on the host. Input DMAs are
consolidated (one per weight, one per 512-seq column block of x^T) and
priority-ordered so the first projections start ~6us in; the causal mask is
sliced out of a single [128, 896] staircase master. O-projection for the
last 512 q columns is split per head-group so one group's 8 matmuls ride as
fillers inside the final attention call; a few dummy matmuls at the top
warm the PE clock (DVFS p-state) while the first DMAs land.
"""

import numpy as np

import concourse.bacc as bacc
import concourse.tile as tile
from concourse import mybir
from concourse.bass_utils import run_bass_kernel_spmd

B, S, D, H = 2, 2048, 1024, 16
HD = D // H          # 64
P = 128
NCORES = 8
GROUPS = 4           # head groups (tensor parallel)
HPG = H // GROUPS    # 4 heads per group
CD = HPG * HD        # 256 local head dims per core
QT = 512             # q tile (matmul free dim)
KT = 128             # kv tile (psum partition dim)
NQT = S // QT        # 4
NKT = S // KT        # 16
KD = D // P          # 8 contraction tiles over the model dim
MW = 3 * KT + QT     # 896: master mask width

F32 = mybir.dt.float32
F16 = mybir.dt.float16
EXP = mybir.ActivationFunctionType.Exp

_NC_CACHE = {}


def _build_nc():
    if "nc" in _NC_CACHE:
        return _NC_CACHE["nc"]
    nc = bacc.Bacc()
    xt = nc.declare_dram_parameter("xt", [NQT, P, KD, QT], F16, isOutput=False)
    wq = nc.declare_dram_parameter("wq", [P, KD, CD], F16, isOutput=False)
    wk = nc.declare_dram_parameter("wk", [P, KD, CD], F16, isOutput=False)
    wv = nc.declare_dram_parameter("wv", [P, KD, CD], F16, isOutput=False)
    wo = nc.declare_dram_parameter("wo", [P, 2, D], F16, isOutput=False)
    bq = nc.declare_dram_parameter("bq", [CD], F32, isOutput=False)
    bk = nc.declare_dram_parameter("bk", [CD], F32, isOutput=False)
    bv = nc.declare_dram_parameter("bv", [HPG, HD], F32, isOutput=False)
    msk = nc.declare_dram_parameter("msk", [P, MW], F16, isOutput=False)
    # o partials: t = 0..2 with both groups summed on-chip; t = 3 kept
    # per-group so the o-projection can overlap the last attention call
    otc = nc.declare_dram_parameter("otc", [NQT - 1, KD, P, QT], F16,
                                    isOutput=True)
    otg = nc.declare_dram_parameter("otg", [2, KD, P, QT], F16, isOutput=True)

    import concourse.bass as bass

    with tile.TileContext(nc) as tc:
        with tc.tile_pool(name="consts", bufs=1) as consts, \
             tc.tile_pool(name="work", bufs=3) as work, \
             tc.tile_pool(name="ps_s", bufs=2, space="PSUM") as ps_s, \
             tc.tile_pool(name="ps_av", bufs=2, space="PSUM") as ps_av, \
             tc.tile_pool(name="ps_po", bufs=2, space="PSUM") as ps_po:

            # ---- constant / persistent SBUF tensors ----
            xt_sb = consts.tile([P, KD, S], F16)
            wq_sb = consts.tile([P, KD, CD], F16)
            wk_sb = consts.tile([P, KD, CD], F16)
            wv_sb = consts.tile([P, KD, CD], F16)
            wo_sb = consts.tile([P, 2, D], F16)
            bq_sb = consts.tile([P, 2], F32)
            bk_sb = consts.tile([P, 2], F32)
            bv_sb = consts.tile([P, HPG, HD], F32)
            msk_sb = consts.tile([P, MW], F16)
            qT_sb = consts.tile([P, 2, S], F16)
            kT_sb = consts.tile([P, 2, S], F16)
            v2_sb = consts.tile([P, NKT, 2, 3 * HD], F16)
            aT_sb = consts.tile([P, 2, NQT, QT], F16)
            dumm_sb = consts.tile([P, QT], F16)

            # ---- PE clock pre-warm: the PE DVFS p-state needs ~3us of
            # continuous execution to reach full clock; run throwaway
            # matmuls on an un-DMA'd tile while the input DMAs stream ----
            nc.vector.memset(dumm_sb, 0.0)
            ps_warm = ps_po.tile([P, QT], F32, tag="po", name="ps_warm")
            for w in range(6):
                nc.tensor.matmul(ps_warm, dumm_sb[:, 0:P], dumm_sb,
                                 start=(w == 0), stop=(w == 5))

            # ---- input DMAs: consolidated, priority-ordered so the first
            # projections (wq + x columns 0:512) can start ~6us in.  Small
            # tensors ride the gpsimd queue in parallel with the sync queue.
            nc.gpsimd.dma_start(out=bq_sb, in_=bq[:].rearrange("(m p) -> p m", p=P))
            nc.gpsimd.dma_start(out=bk_sb, in_=bk[:].rearrange("(m p) -> p m", p=P))
            bv_ap = bv[:, :]
            bv_bc = bass.AP(tensor=bv_ap.tensor, offset=bv_ap.offset,
                            ap=[[0, P]] + list(bv_ap.ap))
            nc.gpsimd.dma_start(out=bv_sb, in_=bv_bc)
            nc.gpsimd.dma_start(out=msk_sb, in_=msk[:, :])
            nc.vector.memset(v2_sb[:, :, :, HD:2 * HD], 1.0)
            # big tensors priority-serial on the sync queue (parallel queues
            # just steal HBM bandwidth from the critical first transfers);
            # wq/xt0 split in halves so the first projection matmuls start
            # as soon as the first half lands (subtile deps)
            h = KD // 2
            nc.sync.dma_start(out=wq_sb[:, 0:h, :], in_=wq[:, 0:h, :])
            nc.sync.dma_start(out=xt_sb[:, 0:h, 0:QT], in_=xt[0][:, 0:h, :])
            nc.sync.dma_start(out=wq_sb[:, h:KD, :], in_=wq[:, h:KD, :])
            nc.sync.dma_start(out=xt_sb[:, h:KD, 0:QT], in_=xt[0][:, h:KD, :])
            nc.sync.dma_start(out=wk_sb, in_=wk[:, :, :])
            nc.sync.dma_start(out=wv_sb, in_=wv[:, :, :])
            nc.sync.dma_start(out=xt_sb[:, :, QT:2 * QT], in_=xt[1])
            nc.sync.dma_start(out=wo_sb, in_=wo[:, :, :])
            nc.sync.dma_start(out=xt_sb[:, :, 2 * QT:3 * QT], in_=xt[2])
            nc.sync.dma_start(out=xt_sb[:, :, 3 * QT:4 * QT], in_=xt[3])

            # ---- helpers ----
            def proj_qk(w_sb, b_sb, dst, mt, nts):
                pss = [ps_po.tile([P, QT], F32, tag="po", name=f"ps_qk{j}")
                       for j in range(len(nts))]
                for kt in range(KD):
                    lhs = w_sb[:, kt, mt * P:(mt + 1) * P]
                    for j, nt in enumerate(nts):
                        nc.tensor.matmul(
                            pss[j], lhs,
                            xt_sb[:, kt, nt * QT:(nt + 1) * QT],
                            start=(kt == 0), stop=(kt == KD - 1))
                for j, nt in enumerate(nts):
                    nc.vector.tensor_scalar_add(
                        dst[:, mt, nt * QT:(nt + 1) * QT], pss[j],
                        b_sb[:, mt:mt + 1])

            def proj_v(jt0, jt1):
                for jt in range(jt0, jt1):
                    ps = ps_po.tile([P, QT], F32, tag="po", name="ps_v")
                    for kt in range(KD):
                        nc.tensor.matmul(
                            ps[:, :CD], xt_sb[:, kt, jt * P:(jt + 1) * P],
                            wv_sb[:, kt, :],
                            start=(kt == 0), stop=(kt == KD - 1))
                    psh = ps[:, :CD].rearrange("p (h d) -> p h d", h=HPG)
                    # even heads -> cols 0:64, odd heads -> cols 128:192
                    nc.vector.tensor_add(
                        v2_sb[:, jt, :, 0:HD], psh[:, 0::2, :], bv_sb[:, 0::2, :])
                    nc.vector.tensor_add(
                        v2_sb[:, jt, :, 2 * HD:3 * HD], psh[:, 1::2, :],
                        bv_sb[:, 1::2, :])

            def attention(t, g, fillers=(), last=False):
                n_kv = 4 * (t + 1)
                # bank A: rows 0:64 = attn h(2g), rows 64:128 = denom h(2g)
                # bank B: rows 0:64 = denom h(2g+1), rows 64:128 = attn h(2g+1)
                av_a = ps_av.tile([P, QT], F32, tag="avden", name="av_a")
                av_b = ps_av.tile([P, QT], F32, tag="avden", name="av_b")

                def vr(kv):
                    r = kv - 4 * t
                    return r, (KT * r if r >= 1 else 0)   # first valid q col

                def scores(kv):
                    r, v0 = vr(kv)
                    s = ps_s.tile([P, 2 * QT], F32, tag="s", name="s")
                    for idx in range(2):
                        p0 = 64 * idx
                        nc.tensor.matmul(
                            s[:, idx * QT + v0:(idx + 1) * QT],
                            kT_sb[p0:p0 + 64, g, kv * KT:(kv + 1) * KT],
                            qT_sb[p0:p0 + 64, g, t * QT + v0:(t + 1) * QT],
                            start=True, stop=True)
                    return s

                # software-pipelined: the PE issues scores(kv+1) BEFORE the
                # exp/mask-dependent AV(kv) so it never idles waiting on the
                # ACT->DVE round trip.  The slot-0 filler may feed scores(0)
                # (k-projection just in time), so it runs before the prologue.
                if fillers and fillers[0] is not None:
                    fillers[0]()
                s = scores(0)
                for kv in range(n_kv):
                    r, v0 = vr(kv)
                    p_t = work.tile([P, 2 * QT], F16, tag="pt", name="p_t")
                    if r < 1:
                        nc.scalar.activation(p_t, s, EXP)
                    else:
                        for idx in range(2):
                            sl = slice(idx * QT + v0, (idx + 1) * QT)
                            nc.scalar.activation(p_t[:, sl], s[:, sl], EXP)
                    if r >= 0:
                        # causal mask for diagonal offset r: slice the
                        # [128, 896] staircase master at column 128*(3-r)
                        mb = KT * (3 - r)
                        for idx in range(2):
                            sl = slice(idx * QT + v0, (idx + 1) * QT)
                            nc.vector.tensor_mul(
                                p_t[:, sl], p_t[:, sl],
                                msk_sb[:, mb + v0:mb + QT])
                    if kv + 1 < n_kv:
                        s = scores(kv + 1)
                    if 0 < kv < len(fillers) and fillers[kv] is not None:
                        fillers[kv]()
                    for idx, bank in ((0, av_a), (1, av_b)):
                        rhs = p_t[:, idx * QT + v0:(idx + 1) * QT]
                        lhsT = v2_sb[:, kv, g, HD * idx:HD * idx + 2 * HD]
                        nc.tensor.matmul(
                            bank[:, v0:], lhsT, rhs,
                            start=(kv == 0), stop=(kv == n_kv - 1))
                # normalize: aT = av / den, denominators shifted across
                # partition halves via a small SBUF->SBUF DMA
                rca = work.tile([P, QT], F32, tag="rca", name="rca")
                rcb = work.tile([P, QT], F32, tag="rcb", name="rcb")
                rc2 = work.tile([P, QT], F32, tag="rc2", name="rc2")
                nc.vector.reciprocal_approx_fast(rca, av_a)
                nc.vector.reciprocal_approx_fast(rcb, av_b)
                # the final call's shifts ride the idle scalar queue so they
                # are not stuck behind output DMAs on the sync queue
                dma_q = nc.scalar if last else nc.sync
                dma_q.dma_start(out=rc2[0:64, :], in_=rca[64:128, :])
                dma_q.dma_start(out=rc2[64:128, :], in_=rcb[0:64, :])
                nc.vector.tensor_mul(aT_sb[0:64, g, t, :], av_a[0:64, :],
                                     rc2[0:64, :])
                nc.vector.tensor_mul(aT_sb[64:128, g, t, :], av_b[64:128, :],
                                     rc2[64:128, :])

            # o-projection into fp16 staging, one DMA per staged tile.
            # t = 0..2: both groups accumulated; t = 3: per-group partials.
            og_st = {}

            def og_unit(t, m0, n_mt=2, g=None, dma=False, copy_alt=False):
                key = (t, g)
                if key not in og_st:
                    og_st[key] = work.tile([P, KD, QT], F16, tag="ogst",
                                           name=f"og{t}{g}")
                st = og_st[key]
                for mt_e in range(m0, m0 + n_mt):
                    ps = ps_po.tile([P, QT], F32, tag="po", name="ps_o")
                    if g is None:
                        for gg in range(2):
                            nc.tensor.matmul(
                                ps, wo_sb[:, gg, mt_e * P:(mt_e + 1) * P],
                                aT_sb[:, gg, t, :],
                                start=(gg == 0), stop=(gg == 1))
                    else:
                        nc.tensor.matmul(
                            ps, wo_sb[:, g, mt_e * P:(mt_e + 1) * P],
                            aT_sb[:, g, t, :], start=True, stop=True)
                    # copy_alt alternates psum->sbuf copies between the
                    # scalar and vector engines (tail: both near-idle)
                    if copy_alt and mt_e % 2 == 0:
                        nc.scalar.copy(st[:, mt_e, :], ps)
                    else:
                        nc.vector.tensor_copy(st[:, mt_e, :], ps)
                if dma:
                    dst = otc[t] if g is None else otg[g]
                    nc.sync.dma_start(
                        out=dst.rearrange("m p c -> p m c"), in_=st)

            # ---- filler-interleaved schedule: the PE stream alternates
            # one projection/o-proj chunk per attention kv-iteration so the
            # in-order PE queue never sits on a block of non-attention work
            # while ACT starves ----
            def fq(mt, nt):
                return lambda: proj_qk(wq_sb, bq_sb, qT_sb, mt, [nt])

            def fk(mt, nt):
                return lambda: proj_qk(wk_sb, bk_sb, kT_sb, mt, [nt])

            def fv(jt):
                return lambda: proj_v(jt, jt + 1)

            def fo(t, m0, g=None, dma=False):
                return lambda: og_unit(t, m0, g=g, dma=dma)

            # prefix: just enough for att(0,0)
            proj_qk(wq_sb, bq_sb, qT_sb, 0, [0])
            proj_qk(wk_sb, bk_sb, kT_sb, 0, [0])
            proj_v(0, 1)
            attention(0, 0, [fv(1), fv(2), fv(3), fq(1, 0)])
            attention(0, 1, [fk(1, 0), fq(0, 1), fk(0, 1), fq(1, 1)])
            attention(1, 0, [fv(4), fo(0, 0), fo(0, 2), fo(0, 4),
                             fo(0, 6, dma=True), fv(5), fv(6), fv(7)])
            attention(1, 1, [fk(1, 1), fq(0, 2), fk(0, 2), fq(1, 2),
                             fk(1, 2)])
            attention(2, 0, [fv(8), fo(1, 0), fo(1, 2), fo(1, 4),
                             fo(1, 6, dma=True), fv(9), fv(10), fv(11)])
            attention(2, 1, [fq(0, 3), fk(0, 3), fv(12), fv(13), fv(14),
                             fv(15), fq(1, 3), fk(1, 3)])
            attention(3, 0, [None, None, fo(2, 0), None, None, fo(2, 2),
                             None, None, fo(2, 4), None, None,
                             fo(2, 6, dma=True)])
            def og30_mid():
                og_unit(3, 2, n_mt=2, g=0)
                nc.sync.dma_start(
                    out=otg[0, 0:4].rearrange("m p c -> p m c"),
                    in_=og_st[(3, 0)][:, 0:4, :])

            attention(3, 1, [None, None, None, None, None, None,
                             fo(3, 0, g=0), None, None, None, None,
                             og30_mid], last=True)
            # tail: og(3,0)'s remaining units run right after the last AV so
            # the PE stays busy (and at full clock) while att(3,1)'s
            # normalize chain (recip -> cross-partition DMA -> mul) drains;
            # og(3,1) then alternates its psum->sbuf copies between the
            # scalar and vector engines with a finer-grained output DMA
            og_unit(3, 4, n_mt=4, g=0)
            nc.sync.dma_start(out=otg[0, 4:8].rearrange("m p c -> p m c"),
                              in_=og_st[(3, 0)][:, 4:8, :])
            og_unit(3, 0, n_mt=4, g=1, copy_alt=True)
            nc.sync.dma_start(out=otg[1, 0:4].rearrange("m p c -> p m c"),
                              in_=og_st[(3, 1)][:, 0:4, :])
            og_unit(3, 4, n_mt=2, g=1, copy_alt=True)
            nc.sync.dma_start(out=otg[1, 4:6].rearrange("m p c -> p m c"),
                              in_=og_st[(3, 1)][:, 4:6, :])
            og_unit(3, 6, n_mt=2, g=1, copy_alt=True)
            nc.sync.dma_start(out=otg[1, 6:8].rearrange("m p c -> p m c"),
                              in_=og_st[(3, 1)][:, 6:8, :])

    nc.compile()
    _NC_CACHE["nc"] = nc
    return nc


def _make_mask():
    # staircase master: M[p, u] = 1 iff p <= u - 384; mask for diagonal
    # offset r (valid iff p <= c - 128 r, c in [0,512)) = M[:, 128(3-r)+c]
    pp = np.arange(P)[:, None]
    uu = np.arange(MW)[None, :]
    return (pp <= uu - 3 * KT).astype(np.float16)


def _in_maps(x, Wq, bq, Wk, bk, Wv, bv, Wo):
    scale = np.float32(1.0 / np.sqrt(HD))
    mask = _make_mask()
    maps = []
    for core in range(NCORES):
        b, g = divmod(core, GROUPS)
        csl = slice(g * CD, (g + 1) * CD)
        xT = np.ascontiguousarray(x[b].T).astype(np.float16)        # [D, S]
        xt4 = np.ascontiguousarray(
            xT.reshape(KD, P, NQT, QT).transpose(2, 1, 0, 3))
        wq_h = np.ascontiguousarray(
            (np.asarray(Wq[:, csl]) * scale).astype(np.float16)
            .reshape(KD, P, CD).transpose(1, 0, 2))
        wk_h = np.ascontiguousarray(
            np.asarray(Wk[:, csl]).astype(np.float16)
            .reshape(KD, P, CD).transpose(1, 0, 2))
        wv_h = np.ascontiguousarray(
            np.asarray(Wv[:, csl]).astype(np.float16)
            .reshape(KD, P, CD).transpose(1, 0, 2))
        wo_h = np.ascontiguousarray(
            np.asarray(Wo[csl, :]).astype(np.float16)
            .reshape(2, P, D).transpose(1, 0, 2))
        maps.append({
            "xt": xt4,
            "wq": wq_h,
            "wk": wk_h,
            "wv": wv_h,
            "wo": wo_h,
            "bq": np.ascontiguousarray(bq[csl] * scale).astype(np.float32),
            "bk": np.ascontiguousarray(bk[csl]).astype(np.float32),
            "bv": np.ascontiguousarray(bv[csl]).reshape(HPG, HD).astype(np.float32),
            "msk": mask,
        })
    return maps


def _ot_to_oT(otc, otg):
    """otc [3, KD, P, QT] + otg [2, KD, P, QT] fp16 -> [D, S] fp32 partial."""
    a = np.asarray(otc).reshape(NQT - 1, KD, P, QT).astype(np.float32)
    g = np.asarray(otg).reshape(2, KD, P, QT).astype(np.float32).sum(axis=0)
    full = np.concatenate([a, g[None]], axis=0)     # [NQT, KD, P, QT]
    return full.transpose(1, 2, 0, 3).reshape(D, S)


def kernel_with_results(x, Wq, bq, Wk, bk, Wv, bv, Wo, bo, trace=False):
    nc = _build_nc()
    maps = _in_maps(x, Wq, bq, Wk, bk, Wv, bv, Wo)
    kwargs = {}
    if trace:
        kwargs = dict(trace=True, trace_cores=[0])
    res = run_bass_kernel_spmd(nc, maps, core_ids=list(range(NCORES)), **kwargs)
    out = np.zeros((B, S, D), dtype=np.float32)
    for b in range(B):
        acc = np.zeros((D, S), dtype=np.float32)
        for g in range(GROUPS):
            r = res.results[b * GROUPS + g]
            acc += _ot_to_oT(r["otc"], r["otg"])
        out[b] = acc.T + np.asarray(bo, dtype=np.float32)[None, :]
    return out, res


def kernel(x, Wq, bq, Wk, bk, Wv, bv, Wo, bo):
    out, _ = kernel_with_results(x, Wq, bq, Wk, bk, Wv, bv, Wo, bo, trace=False)
    return out
